# revision 54
# baseline (speedup 1.0000x reference)
# Trainium2 Bass kernel for nn_BuNNLayer (bundle-rotation GNN layer).
#
# Decomposition (validated vs reference):
#   theta = gelu(x@W1+b1)@W2 + b2 ; R = per-bundle 2x2 rotations from theta
#   h0 = R(x); z0 = h0 @ Wlin          (Wlin commutes with the diffusion)
#   z_k = (-1/k) L z_{k-1}, L = I - P  (4 steps, P = deginv-scaled adjacency)
#   zdif = sum z_k + blin ; out = BN(x + R^T(zdif))
#
# Distribution: nodes are sharded 2500/core across 8 NeuronCores.
# Host work between launches is index bookkeeping + re-sharding only
# (permutation, gather-by-index, replication, transposes, dtype casts);
# all floating-point math that scales with N*C runs on the NeuronCores.
#
# Feature permutation trick: features are reordered on the host as
#   new_f = i*256 + (b>=64)*128 + (b%64)*2 + t   (b bundle, i in-bundle dim,
#   t col)  so the per-bundle 2x2 rotation mixes whole 128-partition chunks
#   (chunk 0 <-> chunk 2, chunk 1 <-> chunk 3) with cos/sin tiles that are
#   theta duplicated x2 along partitions. No partition-strided swizzles.
#
# Diffusion steps: the host pre-gathers source rows into a pass-major
# stream g (indeg-sorted prefix passes, 128-aligned), the device streams
# g sequentially and does bf16 segment adds on the vector engine.

import sys, types
import numpy as np

for p in ('/opt/trn_rl_repo', '/root/.axon_site'):
    if p not in sys.path:
        sys.path.insert(0, p)

import ml_dtypes
import concourse.bass as bass
import concourse.bacc as bacc
import concourse.mybir as mybir
from concourse.bass_utils import run_bass_kernel_spmd

BF16 = ml_dtypes.bfloat16

N, C = 20000, 512
E_RAND = 140000
GNN = 512
MAX_DEG = 4
TAU = 1.0
EPS = 1e-5
NCORES = 8
NPC = 2500                   # real nodes per core
SLAB = 2560                  # 128-aligned slab
SR = SLAB // 128             # 20 slot rows
NT = 5                       # node tiles in feature-major phases
NTW = 500                    # node tile width
HALFPI = float(np.pi / 2)

_trace = [False]             # set by test harness to collect exec times
_exec_times = []


def _install_ntff_shim():
    try:
        import antenv.axon_hooks  # noqa: F401
        return
    except ImportError:
        pass
    try:
        from trn_agent_boot.trn_boot import _ntff_profile_via_ctypes
        hook = _ntff_profile_via_ctypes('/opt/axon/libaxon_pjrt.so')
    except Exception:
        hook = None
    mod = types.ModuleType("antenv.axon_hooks")
    mod.get_axon_ntff_profile_hook = lambda: hook
    try:
        import antenv  # noqa: F401
    except ImportError:
        pkg = types.ModuleType("antenv")
        pkg.__path__ = []
        sys.modules["antenv"] = pkg
    sys.modules["antenv.axon_hooks"] = mod


def _run(nc, in_maps, tag):
    kw = {}
    if _trace[0]:
        import tempfile
        _install_ntff_shim()
        kw = dict(trace=True, tmpdir=tempfile.mkdtemp(prefix=f"bunn_{tag}_"))
    res = run_bass_kernel_spmd(nc, in_maps, list(range(NCORES)), **kw)
    if _trace[0] and res.exec_time_ns is not None:
        _exec_times.append((tag, res.exec_time_ns))
    return res.results


# ---------------------------------------------------------------- phase A ---
def build_phase_a():
    nc = bacc.Bacc(None, target_bir_lowering=False)
    dt = mybir.dt
    xT = nc.dram_tensor("xT", [128, 4, 2500], dt.bfloat16, kind="ExternalInput")
    xT8 = nc.dram_tensor("xT8", [128, 4, 2500], dt.float8e4, kind="ExternalInput")
    W1 = nc.dram_tensor("W1", [128, 4, 512], dt.float8e4, kind="ExternalInput")
    W2A = nc.dram_tensor("W2A", [128, 4, 128], dt.float8e4, kind="ExternalInput")
    W2B = nc.dram_tensor("W2B", [128, 4, 128], dt.float8e4, kind="ExternalInput")
    WL = nc.dram_tensor("WL", [128, 4, 512], dt.float8e4, kind="ExternalInput")
    b1 = nc.dram_tensor("b1", [128, 4, 1], dt.float32, kind="ExternalInput")
    # per-half b2 biases in duplicated layout + pi/2 and zero constants
    sbA = nc.dram_tensor("sbA", [128, 1], dt.float32, kind="ExternalInput")
    sbB = nc.dram_tensor("sbB", [128, 1], dt.float32, kind="ExternalInput")
    hpi = nc.dram_tensor("hpi", [128, 1], dt.float32, kind="ExternalInput")
    zer = nc.dram_tensor("zer", [128, 1], dt.float32, kind="ExternalInput")
    bl = nc.dram_tensor("bl", [128, 4, 1], dt.float32, kind="ExternalInput")
    z0T = nc.dram_tensor("z0T", [4, 128, 2500], dt.bfloat16, kind="ExternalOutput")
    z0bT = nc.dram_tensor("z0bT", [128, 4, 2500], dt.bfloat16, kind="ExternalOutput")
    thT = nc.dram_tensor("thT", [128, 2, 2500], dt.bfloat16, kind="ExternalOutput")

    import concourse.tile as tile
    with tile.TileContext(nc) as tc:
        with (
            tc.tile_pool(name="cst", bufs=1) as cst,
            tc.tile_pool(name="big", bufs=1) as big,
            tc.tile_pool(name="sm", bufs=2) as sm,
            tc.tile_pool(name="ps", bufs=2, space="PSUM") as ps,
            tc.tile_pool(name="ps2", bufs=2, space="PSUM") as ps2,
        ):
            xt = big.tile([128, 4, 2500], dt.bfloat16)
            xt8 = big.tile([128, 4, 2500], dt.float8e4)
            w1 = cst.tile([128, 4, 512], dt.float8e4)
            w2a = cst.tile([128, 4, 128], dt.float8e4)
            w2b = cst.tile([128, 4, 128], dt.float8e4)
            wl = cst.tile([128, 4, 512], dt.float8e4)
            b1t = cst.tile([128, 4, 1], dt.float32)
            sbias = {}
            for nm, src in (("sbA", sbA), ("sbB", sbB),
                            ("hpi", hpi), ("zer", zer)):
                t = cst.tile([128, 1], dt.float32, tag=f"sb_{nm}")
                nc.sync.dma_start(t[:], src[:])
                sbias[nm] = t
            blt = cst.tile([128, 4, 1], dt.float32)
            nc.sync.dma_start(w1[:], W1[:])
            nc.sync.dma_start(b1t[:], b1[:])
            nc.sync.dma_start(xt8[:], xT8[:])
            nc.sync.dma_start(xt[:], xT[:])
            nc.sync.dma_start(w2a[:], W2A[:])
            nc.sync.dma_start(w2b[:], W2B[:])
            nc.sync.dma_start(wl[:], WL[:])
            nc.sync.dma_start(blt[:], bl[:])

            t1 = big.tile([128, 4, 2500], dt.float8e4)
            cosc = big.tile([128, 2, 2500], dt.bfloat16)
            sinc = big.tile([128, 2, 2500], dt.bfloat16)
            sinn = big.tile([128, 2, 2500], dt.bfloat16)
            thab = big.tile([128, 2, 2500], dt.bfloat16)

            DR = mybir.MatmulPerfMode.DoubleRow
            for nt in range(NT):
                ns = slice(nt * NTW, (nt + 1) * NTW)
                # t1 = gelu(x @ W1 + b1): fp8 DoubleRow, K=256 per matmul
                for gc in range(4):
                    pt = ps.tile([128, NTW], dt.float32)
                    for i in range(2):
                        nc.tensor.matmul(
                            pt[:], w1[:, 2 * i:2 * i + 2, gc * 128:(gc + 1) * 128],
                            xt8[:, 2 * i:2 * i + 2, ns],
                            start=(i == 0), stop=(i == 1), perf_mode=DR)
                    nc.scalar.activation(
                        t1[:, gc, ns], pt[:],
                        mybir.ActivationFunctionType.Gelu,
                        bias=b1t[:, gc, :], scale=1.0)
                # theta in duplicated layout, halves A (bundles 0:64) and
                # B (bundles 64:128): partition p <- bundle p//2
                for h, (w2h, sb) in enumerate(((w2a, "sbA"), (w2b, "sbB"))):
                    pt = ps.tile([128, NTW], dt.float32)
                    for i in range(2):
                        nc.tensor.matmul(pt[:], w2h[:, 2 * i:2 * i + 2, :],
                                         t1[:, 2 * i:2 * i + 2, ns],
                                         start=(i == 0), stop=(i == 1),
                                         perf_mode=DR)
                    nc.vector.tensor_scalar_add(thab[:, h, ns], pt[:],
                                                sbias[sb][:])
                # cos/sin for this node tile (theta includes b2): one
                # [128, 2, NTW] op per function keeps the rot pipeline fed
                nc.scalar.activation(cosc[:, :, ns], thab[:, :, ns],
                                     mybir.ActivationFunctionType.Sin,
                                     bias=sbias["hpi"][:], scale=1.0)
                nc.scalar.activation(sinc[:, :, ns], thab[:, :, ns],
                                     mybir.ActivationFunctionType.Sin,
                                     bias=sbias["zer"][:], scale=1.0)
                nc.scalar.activation(sinn[:, :, ns], thab[:, :, ns],
                                     mybir.ActivationFunctionType.Sin,
                                     bias=sbias["zer"][:], scale=-1.0)
            nc.sync.dma_start(thT[:], thab[:])

            # rotation + z0 GEMM, pipelined per node tile
            # R: out_i0 = c*v0 - s*v1 ; out_i1 = s*v0 + c*v1
            # half h: v0 = chunk h, v1 = chunk 2+h
            h0b = big.tile([128, 4, 2500], dt.float8e4)
            for nt in range(NT):
                ns = slice(nt * NTW, (nt + 1) * NTW)
                for h in range(2):
                    q0, q2 = h, 2 + h
                    tm1 = sm.tile([128, NTW], dt.bfloat16, tag="tm1")
                    tm2 = sm.tile([128, NTW], dt.bfloat16, tag="tm2")
                    nc.vector.tensor_tensor(tm1[:], cosc[:, h, ns], xt[:, q0, ns],
                                            op=mybir.AluOpType.mult)
                    nc.vector.tensor_tensor(tm2[:], sinn[:, h, ns], xt[:, q2, ns],
                                            op=mybir.AluOpType.mult)
                    nc.vector.tensor_tensor(h0b[:, q0, ns], tm1[:], tm2[:],
                                            op=mybir.AluOpType.add)
                    tm3 = sm.tile([128, NTW], dt.bfloat16, tag="tm1")
                    tm4 = sm.tile([128, NTW], dt.bfloat16, tag="tm2")
                    nc.vector.tensor_tensor(tm3[:], sinc[:, h, ns], xt[:, q0, ns],
                                            op=mybir.AluOpType.mult)
                    nc.vector.tensor_tensor(tm4[:], cosc[:, h, ns], xt[:, q2, ns],
                                            op=mybir.AluOpType.mult)
                    nc.vector.tensor_tensor(h0b[:, q2, ns], tm3[:], tm4[:],
                                            op=mybir.AluOpType.add)
                # z0 = h0 @ Wlin -> bf16 out  (fp8 DoubleRow)
                for mc in range(4):
                    pt = ps2.tile([128, NTW], dt.float32)
                    for i in range(2):
                        nc.tensor.matmul(
                            pt[:], wl[:, 2 * i:2 * i + 2, mc * 128:(mc + 1) * 128],
                            h0b[:, 2 * i:2 * i + 2, ns],
                            start=(i == 0), stop=(i == 1), perf_mode=DR)
                    z0s = sm.tile([128, NTW], dt.bfloat16, tag="z0s")
                    nc.vector.tensor_copy(z0s[:], pt[:])
                    nc.sync.dma_start(z0T[mc, :, ns], z0s[:])
                    # z0b = z0 + blin: phase C's zT[0] term with the bias
                    # pre-added (blin must NOT enter the diffusion input)
                    z0bs = sm.tile([128, NTW], dt.bfloat16, tag="z0bs")
                    nc.vector.tensor_scalar_add(z0bs[:], z0s[:], blt[:, mc, :])
                    nc.sync.dma_start(z0bT[:, mc, ns], z0bs[:])
    nc.finalize()
    return nc


# ---------------------------------------------------------------- phase B ---
PE_ROWS = 8  # agg slot rows accumulated on the tensor engine (PSUM banks)


def build_phase_b(lo_tot, hi_tot, lo_cnts, hi_cnts, pass_info):
    """One diffusion step, host-pre-gathered message stream.

    The message stream is split: rows 0..PE_ROWS of every pass stream as
    fp8 (glo) and accumulate exactly via identity matmuls into fp32 PSUM
    banks (tensor engine); rows PE_ROWS.. stream as bf16 (ghi) and
    accumulate via 2x-mode DVE adds. The deginv scaling rides ACT-engine
    evictions; every agg row is finalized and written out as soon as the
    last pass touching it completes.

    lo_cnts/hi_cnts: per-chunk row counts of the two streams
    pass_info: (chunk_idx, lo_off, hi_off, rows) per pass, in order;
       pass 0 must cover all SR rows.
    """
    nc = bacc.Bacc(None, target_bir_lowering=False)
    dt = mybir.dt
    glo = nc.dram_tensor("glo", [128, lo_tot, 512], dt.float8e4, kind="ExternalInput")
    ghi = nc.dram_tensor("ghi", [128, hi_tot, 512], dt.float8e4, kind="ExternalInput")
    cur = nc.dram_tensor("cur", [128, SR, 512], dt.bfloat16, kind="ExternalInput")
    dgi = nc.dram_tensor("dgi", [128, SR], dt.float32, kind="ExternalInput")
    alp = nc.dram_tensor("alp", [128, 1], dt.float32, kind="ExternalInput")
    id8 = nc.dram_tensor("id8", [128, 128], dt.float8e4, kind="ExternalInput")
    idb = nc.dram_tensor("idb", [128, 128], dt.bfloat16, kind="ExternalInput")
    out = nc.dram_tensor("out", [128, SR, 512], dt.bfloat16, kind="ExternalOutput")
    HI = SR - PE_ROWS
    GLOMAX = max(lo_cnts)
    GHIMAX = max(hi_cnts) if hi_tot else 0
    # matmul count per PSUM row (cur seed + passes covering it)
    nblk = [1 + sum(1 for (_, _, _, r) in pass_info if r > j)
            for j in range(PE_ROWS)]
    # last pass index touching each agg row
    last_pass = {}
    for j in range(SR):
        lp = max(i for i, (_, _, _, r) in enumerate(pass_info) if r > j)
        last_pass.setdefault(lp, []).append(j)

    import concourse.tile as tile
    with tile.TileContext(nc) as tc:
        with (
            tc.tile_pool(name="cst", bufs=1) as cst,
            tc.tile_pool(name="gb", bufs=2) as gb,
            tc.tile_pool(name="big", bufs=1) as big,
            tc.tile_pool(name="pp", bufs=1, space="PSUM") as pp,
        ):
            dgit = cst.tile([128, SR], dt.float32)
            alpt = cst.tile([128, 1], dt.float32)
            id8t = cst.tile([128, 128], dt.float8e4)
            idbt = cst.tile([128, 128], dt.bfloat16)
            curt = big.tile([128, SR, 512], dt.bfloat16)
            nc.sync.dma_start(id8t[:], id8[:])
            nc.sync.dma_start(idbt[:], idb[:])
            nc.sync.dma_start(dgit[:], dgi[:])
            nc.sync.dma_start(alpt[:], alp[:])
            # lo rows first: the PSUM seed matmuls consume them earliest
            nc.sync.dma_start(curt[:, :PE_ROWS, :], cur[:, :PE_ROWS, :])
            nc.sync.dma_start(curt[:, PE_ROWS:, :], cur[:, PE_ROWS:, :])

            # ct = alpha*cur, independent of the message stream
            ct = big.tile([128, SR, 512], dt.bfloat16)
            nc.vector.tensor_scalar_mul(ct[:], curt[:], alpt[:])

            # PSUM accumulators for rows 0..PE_ROWS, seeded with cur rows
            # (seed emitted first with start=True: accumulation-group order
            # within a bank must match program order)
            pts = [pp.tile([128, 512], dt.float32, tag=f"pb{j}", name=f"pb{j}")
                   for j in range(PE_ROWS)]
            cnt = [0] * PE_ROWS
            for j in range(PE_ROWS):
                nc.tensor.matmul(pts[j][:], idbt[:], curt[:, j, :],
                                 start=True, stop=(nblk[j] == 1))
                cnt[j] = 1

            agg_hi = big.tile([128, HI, 512], dt.bfloat16)
            mt = big.tile([128, SR, 512], dt.bfloat16)
            ot = big.tile([128, SR, 512], dt.bfloat16)
            first_hi = True
            lo0 = hi0 = 0
            for ci in range(len(lo_cnts)):
                lc, hc = lo_cnts[ci], hi_cnts[ci]
                lt = gb.tile([128, max(GLOMAX, 1), 512], dt.float8e4, tag="glo")
                nc.sync.dma_start(lt[:, :lc, :], glo[:, lo0:lo0 + lc, :])
                if hc:
                    # hi rows stream as fp8; ACT upcasts for the 2x DVE adds
                    ht8 = gb.tile([128, max(GHIMAX, 1), 512], dt.float8e4,
                                  tag="ghi8")
                    nc.sync.dma_start(ht8[:, :hc, :], ghi[:, hi0:hi0 + hc, :])
                    ht = gb.tile([128, max(GHIMAX, 1), 512], dt.bfloat16,
                                 tag="ghi")
                    nc.scalar.activation(ht[:, :hc, :], ht8[:, :hc, :],
                                         mybir.ActivationFunctionType.Copy)
                for pi, (cj, lof, hif, rows) in enumerate(pass_info):
                    if cj != ci:
                        continue
                    for j in range(min(rows, PE_ROWS)):
                        cnt[j] += 1
                        nc.tensor.matmul(pts[j][:], id8t[:], lt[:, lof + j, :],
                                         start=False, stop=(cnt[j] == nblk[j]))
                    if rows > PE_ROWS:
                        hi = rows - PE_ROWS
                        if first_hi:
                            nc.vector.tensor_tensor(
                                agg_hi[:], curt[:, PE_ROWS:, :],
                                ht[:, hif:hif + hi, :],
                                op=mybir.AluOpType.add)
                            first_hi = False
                        else:
                            nc.vector.tensor_tensor(
                                agg_hi[:, :hi, :], agg_hi[:, :hi, :],
                                ht[:, hif:hif + hi, :],
                                op=mybir.AluOpType.add)
                    # finalize rows whose last pass just ran:
                    # mt = (alpha*deginv)*agg  (ACT), ot = ct - mt (DVE)
                    if pi in last_pass:
                        rows_f = last_pass[pi]
                        j0, j1 = min(rows_f), max(rows_f) + 1
                        for j in range(j0, j1):
                            src = (pts[j][:] if j < PE_ROWS
                                   else agg_hi[:, j - PE_ROWS, :])
                            nc.scalar.activation(
                                mt[:, j, :], src,
                                mybir.ActivationFunctionType.Copy,
                                scale=dgit[:, j:j + 1])
                        nc.vector.tensor_tensor(
                            ot[:, j0:j1, :], ct[:, j0:j1, :], mt[:, j0:j1, :],
                            op=mybir.AluOpType.subtract)
                        nc.sync.dma_start(out[:, j0:j1, :], ot[:, j0:j1, :])
                lo0 += lc
                hi0 += hc
    nc.finalize()
    return nc


# ---------------------------------------------------------------- phase C ---
NW, CW = 5, 500  # column windows


def build_phase_c():
    nc = bacc.Bacc(None, target_bir_lowering=False)
    dt = mybir.dt
    zT = nc.dram_tensor("zT", [2, NW, 128, 4, CW], dt.bfloat16, kind="ExternalInput")
    z8T = nc.dram_tensor("z8T", [3, NW, 128, 4, CW], dt.float8e4, kind="ExternalInput")
    id8 = nc.dram_tensor("id8", [128, 128], dt.float8e4, kind="ExternalInput")
    thT = nc.dram_tensor("thT", [128, 2, 2500], dt.bfloat16, kind="ExternalInput")
    xT = nc.dram_tensor("xT", [NW, 128, 4, CW], dt.bfloat16, kind="ExternalInput")
    cb2 = nc.dram_tensor("cb2", [128, 1], dt.float32, kind="ExternalInput")
    zb2 = nc.dram_tensor("zb2", [128, 1], dt.float32, kind="ExternalInput")
    idn = nc.dram_tensor("idn", [128, 128], dt.bfloat16, kind="ExternalInput")
    hbnT = nc.dram_tensor("hbnT", [NW, 128, 4, CW], dt.bfloat16, kind="ExternalOutput")
    stats = nc.dram_tensor("stats", [128, 8, NW], dt.float32, kind="ExternalOutput")

    import concourse.tile as tile
    with tile.TileContext(nc) as tc:
        with (
            tc.tile_pool(name="cst", bufs=1) as cst,
            tc.tile_pool(name="big", bufs=1) as big,
            tc.tile_pool(name="zw", bufs=3) as zwp,
            tc.tile_pool(name="sm", bufs=8) as sm,
            tc.tile_pool(name="pp", bufs=2, space="PSUM") as pp,
        ):
            tht = big.tile([128, 2, 2500], dt.bfloat16)
            cb2t = cst.tile([128, 1], dt.float32, tag="cb2")
            zb2t = cst.tile([128, 1], dt.float32, tag="zb2")
            idnt = cst.tile([128, 128], dt.bfloat16)
            id8t = cst.tile([128, 128], dt.float8e4)
            nc.sync.dma_start(tht[:], thT[:])
            nc.sync.dma_start(cb2t[:], cb2[:])
            nc.sync.dma_start(zb2t[:], zb2[:])
            nc.sync.dma_start(idnt[:], idn[:])
            nc.sync.dma_start(id8t[:], id8[:])

            # cos/sin from theta upfront (theta already includes b2)
            cosc = big.tile([128, 2, 2500], dt.bfloat16)
            sinc = big.tile([128, 2, 2500], dt.bfloat16)
            sinn = big.tile([128, 2, 2500], dt.bfloat16)
            nc.scalar.activation(cosc[:], tht[:],
                                 mybir.ActivationFunctionType.Sin,
                                 bias=cb2t[:], scale=1.0)
            nc.scalar.activation(sinc[:], tht[:],
                                 mybir.ActivationFunctionType.Sin,
                                 bias=zb2t[:], scale=1.0)
            nc.scalar.activation(sinn[:], tht[:],
                                 mybir.ActivationFunctionType.Sin,
                                 bias=zb2t[:], scale=-1.0)

            st = big.tile([128, 8, NW], dt.float32)
            for w in range(NW):
                ns = slice(w * CW, (w + 1) * CW)
                zw = zwp.tile([128, 2, 4, CW], dt.bfloat16, tag="zw")
                for k in range(2):
                    nc.sync.dma_start(zw[:, k, :, :], zT[k, w])
                zw8 = zwp.tile([128, 3, 4, CW], dt.float8e4, tag="zw8")
                for k in range(3):
                    nc.sync.dma_start(zw8[:, k, :, :], z8T[k, w])
                xw = zwp.tile([128, 4, CW], dt.bfloat16, tag="xw")
                nc.sync.dma_start(xw[:], xT[w])

                # z = z0b + sum cur_k  via identity-matmul PSUM accumulation
                # (z0, z1 in bf16; the small z2..z4 terms in fp8)
                zs = zwp.tile([128, 4, CW], dt.bfloat16, tag="zs")
                for q in range(4):
                    ptq = pp.tile([128, CW], dt.float32, tag=f"pq{q}")
                    for k in range(2):
                        nc.tensor.matmul(ptq[:], idnt[:], zw[:, k, q, :],
                                         start=(k == 0), stop=False)
                    for k in range(3):
                        nc.tensor.matmul(ptq[:], id8t[:], zw8[:, k, q, :],
                                         start=False, stop=(k == 2))
                    nc.vector.tensor_copy(zs[:, q, :], ptq[:])

                # R^T rotation + x residual + BN partial stats
                for h in range(2):
                    q0, q2 = h, 2 + h
                    for (qo, ca, sa, zb) in ((q0, cosc, sinc, q2),
                                             (q2, cosc, sinn, q0)):
                        tm1 = sm.tile([128, CW], dt.bfloat16, tag="tm1")
                        tm2 = sm.tile([128, CW], dt.bfloat16, tag="tm2")
                        hbq = sm.tile([128, CW], dt.bfloat16, tag="hbq")
                        scr = sm.tile([128, CW], dt.bfloat16, tag="scr")
                        nc.vector.tensor_tensor(tm1[:], ca[:, h, ns],
                                                zs[:, qo, :],
                                                op=mybir.AluOpType.mult)
                        nc.vector.tensor_tensor(tm2[:], sa[:, h, ns],
                                                zs[:, zb, :],
                                                op=mybir.AluOpType.mult)
                        nc.vector.tensor_tensor(tm1[:], tm1[:], tm2[:],
                                                op=mybir.AluOpType.add)
                        nc.vector.tensor_tensor(hbq[:], tm1[:], xw[:, qo, :],
                                                op=mybir.AluOpType.add)
                        nc.scalar.activation(scr[:], hbq[:],
                                             mybir.ActivationFunctionType.Copy,
                                             accum_out=st[:, 2 * qo, w:w + 1])
                        nc.scalar.activation(scr[:], hbq[:],
                                             mybir.ActivationFunctionType.Square,
                                             accum_out=st[:, 2 * qo + 1, w:w + 1])
                        nc.sync.dma_start(hbnT[w, :, qo, :], hbq[:])
            nc.sync.dma_start(stats[:], st[:])
    nc.finalize()
    return nc


# ---------------------------------------------------------------- phase D ---
def build_phase_d():
    nc = bacc.Bacc(None, target_bir_lowering=False)
    dt = mybir.dt
    hbnT = nc.dram_tensor("hbnT", [NW, 128, 4, CW], dt.bfloat16, kind="ExternalInput")
    sc = nc.dram_tensor("sc", [128, 4, 1], dt.float32, kind="ExternalInput")
    sh = nc.dram_tensor("sh", [128, 4, 1], dt.float32, kind="ExternalInput")
    outT = nc.dram_tensor("outT", [NW, 128, 4, CW], dt.bfloat16, kind="ExternalOutput")
    import concourse.tile as tile
    with tile.TileContext(nc) as tc:
        with (
            tc.tile_pool(name="big", bufs=1) as big,
            tc.tile_pool(name="sm", bufs=3) as sm,
        ):
            sct = big.tile([128, 4, 1], dt.float32)
            sht = big.tile([128, 4, 1], dt.float32)
            nc.sync.dma_start(sct[:], sc[:])
            nc.sync.dma_start(sht[:], sh[:])
            for w in range(NW):
                hq = sm.tile([128, 4, CW], dt.bfloat16, tag="hq")
                oq = sm.tile([128, 4, CW], dt.bfloat16, tag="oq")
                nc.sync.dma_start(hq[:], hbnT[w])
                for q in range(4):
                    nc.vector.tensor_scalar(oq[:, q, :], hq[:, q, :],
                                            sct[:, q, :], sht[:, q, :],
                                            op0=mybir.AluOpType.mult,
                                            op1=mybir.AluOpType.add)
                nc.sync.dma_start(outT[w], oq[:])
    nc.finalize()
    return nc


# ------------------------------------------------------------------- host ---
def _feat_perm():
    """new feature order: nf = i*256 + (b>=64)*128 + (b%64)*2 + t"""
    nf = np.arange(C)
    i = nf // 256
    rem = nf % 256
    hb = rem // 128
    pp = rem % 128
    b = hb * 64 + pp // 2
    t = pp % 2
    return (4 * b + 2 * i + t).astype(np.int64)


def _fmajor(nm2500x512_bf16):
    """node-major [2500, 512] -> device feature-major [128, 4, 2500]"""
    return np.ascontiguousarray(
        nm2500x512_bf16.T.reshape(4, 128, 2500).transpose(1, 0, 2))


def _fmajor_w(nm2500x512_bf16):
    """node-major [2500, 512] -> windowed feature-major [NW, 128, 4, CW]"""
    return np.ascontiguousarray(
        nm2500x512_bf16.T.reshape(4, 128, NW, CW).transpose(2, 1, 0, 3))


def _slabify(rows_2500x512, dtype):
    """[2500, 512] node rows -> [128, SR, 512] slab layout (slot s at
    [s%128, s//128]), zero padded."""
    a = np.zeros((SLAB, 512), dtype)
    a[:NPC] = rows_2500x512
    return np.ascontiguousarray(a.reshape(SR, 128, 512).transpose(1, 0, 2))


def kernel(x, W1, b1, W2, b2, Wlin, blin, gamma, beta, edge_index):
    x = np.asarray(x, np.float32)
    ei = np.asarray(edge_index)
    src = ei[0].astype(np.int64)
    dst = ei[1].astype(np.int64)
    rsrc, rdst = src[:E_RAND], dst[:E_RAND]

    deg = np.bincount(src, minlength=N).astype(np.float64)
    deginv = (1.0 / deg).astype(np.float32)
    indeg = np.bincount(rdst, minlength=N)

    pf = _feat_perm()
    x_p = x[:, pf]
    W1p = np.asarray(W1, np.float32)[pf, :]
    Wlp = np.asarray(Wlin, np.float32)[np.ix_(pf, pf)]
    blp = np.asarray(blin, np.float32)[pf]
    gap = np.asarray(gamma, np.float32)[pf]
    bep = np.asarray(beta, np.float32)[pf]
    dup = np.arange(128) // 2
    b2v = np.asarray(b2, np.float32)
    W2A = np.asarray(W2, np.float32)[:, dup]
    W2B = np.asarray(W2, np.float32)[:, 64 + dup]
    b2A = b2v[dup]
    b2B = b2v[64 + dup]

    # ---- node -> (core, slot) assignment: per core, sort by indeg desc ----
    perm_slab = np.empty((NCORES, NPC), np.int64)
    for c in range(NCORES):
        own = np.arange(NPC * c, NPC * (c + 1))
        perm_slab[c] = own[np.argsort(-indeg[own], kind='stable')]

    # pass structure: n_r = max over cores of roundup(#nodes with indeg>r, 128)
    max_d = int(indeg.max())
    n_r = []
    for r in range(max_d):
        m = max(int((indeg[perm_slab[c]] > r).sum()) for c in range(NCORES))
        if m == 0:
            break
        n_r.append(max(int(-(-m // 128) * 128), SLAB if r == 0 else 0))
    assert n_r[0] == SLAB

    # group passes into DMA chunks (fp8 lo rows + bf16 hi rows per pass),
    # capped by per-partition chunk bytes
    rows_r = [nr // 128 for nr in n_r]
    chunks, curch, curb = [], [], 0
    for i, r in enumerate(rows_r):
        b = min(r, PE_ROWS) * 512 + max(r - PE_ROWS, 0) * 1024
        if curch and curb + b > 20 * 1024:
            chunks.append(curch)
            curch, curb = [], 0
        curch.append(i)
        curb += b
    chunks.append(curch)
    pass_info = []
    lo_cnts, hi_cnts = [], []
    for ci, pl in enumerate(chunks):
        lo_off = hi_off = 0
        for i in pl:
            r = rows_r[i]
            pass_info.append((ci, lo_off, hi_off, r))
            lo_off += min(r, PE_ROWS)
            hi_off += max(r - PE_ROWS, 0)
        lo_cnts.append(lo_off)
        hi_cnts.append(hi_off)
    lo_tot, hi_tot = sum(lo_cnts), sum(hi_cnts)

    # CSR of random edges by dst
    order = np.argsort(rdst, kind='stable')
    ssrc = rsrc[order]
    starts = np.zeros(N + 1, np.int64)
    starts[1:] = np.cumsum(np.bincount(rdst, minlength=N))

    # gather source index per (core, stream slot); N = zero row sentinel
    gsrc_lo = np.full((NCORES, lo_tot * 128), N, np.int32)
    gsrc_hi = np.full((NCORES, max(hi_tot, 1) * 128), N, np.int32)
    for c in range(NCORES):
        nodes = perm_slab[c]
        blo = bhi = 0
        for r, nrr in enumerate(rows_r):
            lo = min(nrr, PE_ROWS)
            m = indeg[nodes] > r
            idxs = np.nonzero(m)[0]
            vals = ssrc[starts[nodes[m]] + r]
            lom = idxs < lo * 128
            gsrc_lo[c, blo * 128 + idxs[lom]] = vals[lom]
            if nrr > lo:
                gsrc_hi[c, bhi * 128 + (idxs[~lom] - lo * 128)] = vals[~lom]
            blo += lo
            bhi += nrr - lo

    # per-core slab aux
    dgi_slab = np.zeros((NCORES, SLAB), np.float32)
    for c in range(NCORES):
        dgi_slab[c, :NPC] = deginv[perm_slab[c]]

    # ---------------- phase A ----------------
    F8 = np.dtype(ml_dtypes.float8_e4m3)
    nc_a = build_phase_a()
    W1b = np.ascontiguousarray(
        W1p.astype(F8).reshape(4, 128, GNN).transpose(1, 0, 2))
    W2Ab = np.ascontiguousarray(
        W2A.astype(F8).reshape(4, 128, 128).transpose(1, 0, 2))
    W2Bb = np.ascontiguousarray(
        W2B.astype(F8).reshape(4, 128, 128).transpose(1, 0, 2))
    WLb = np.ascontiguousarray(
        Wlp.astype(F8).reshape(4, 128, C).transpose(1, 0, 2))
    b1b = np.ascontiguousarray(
        np.asarray(b1, np.float32).reshape(4, 128, 1).transpose(1, 0, 2))
    blb = np.ascontiguousarray(blp.reshape(4, 128, 1).transpose(1, 0, 2))
    ident = np.eye(128, dtype=BF16)
    in_a = []
    xT_cores = []
    for c in range(NCORES):
        xTc = _fmajor(x_p[perm_slab[c]].astype(BF16))
        xT_cores.append(xTc)
        in_a.append(dict(
            xT=xTc, xT8=xTc.astype(F8),
            W1=W1b, W2A=W2Ab, W2B=W2Bb, WL=WLb, b1=b1b, bl=blb,
            sbA=b2A.reshape(128, 1), sbB=b2B.reshape(128, 1),
            hpi=np.full((128, 1), HALFPI, np.float32),
            zer=np.zeros((128, 1), np.float32)))
    res_a = _run(nc_a, in_a, "A")
    # z0 node-major fp32 per core; theta kept for phase C
    z0_nm = []
    theta_out = []
    for c in range(NCORES):
        z0_nm.append(np.asarray(res_a[c]["z0T"]).reshape(C, NPC).T
                     .astype(np.float32))
        theta_out.append(np.asarray(res_a[c]["thT"]))
    if _trace[0]:
        z0a = np.stack(z0_nm)
        print(f"[dbg] z0: |z0|={np.abs(z0a).max():.4g} rms={z0a.std():.4g}")

    # ---------------- phase B x 4 ----------------
    nc_b = build_phase_b(lo_tot, hi_tot, lo_cnts, hi_cnts, pass_info)
    ident8 = np.eye(128, dtype=F8)
    # cur state by original node id
    curp = np.zeros((N + 1, C), BF16)
    for c in range(NCORES):
        curp[perm_slab[c]] = z0_nm[c].astype(BF16)
    slab_idx = [np.concatenate([perm_slab[c], np.full(SLAB - NPC, N)])
                for c in range(NCORES)]
    cur_fm = []  # per-step, per-core feature-major bf16 cur terms for phase C
    for k in range(1, MAX_DEG + 1):
        alpha = np.float32(-TAU / k)
        curp8 = curp.astype(F8)
        in_b = []
        for c in range(NCORES):
            glo_in = np.ascontiguousarray(
                curp8[gsrc_lo[c]].reshape(lo_tot, 128, C).transpose(1, 0, 2))
            ghi_in = np.ascontiguousarray(
                curp8[gsrc_hi[c]].reshape(max(hi_tot, 1), 128, C)
                .transpose(1, 0, 2))
            cur_in = np.ascontiguousarray(
                curp[slab_idx[c]].reshape(SR, 128, C).transpose(1, 0, 2))
            dgi2 = np.ascontiguousarray(
                (alpha * dgi_slab[c]).reshape(SR, 128).T)
            in_b.append(dict(glo=glo_in, ghi=ghi_in, cur=cur_in, dgi=dgi2,
                             id8=ident8, idb=ident,
                             alp=np.full((128, 1), alpha, np.float32)))
        res_b = _run(nc_b, in_b, f"B{k}")
        terms = []
        for c in range(NCORES):
            o = np.asarray(res_b[c]["out"])  # [128, SR, 512] bf16
            nm = o.transpose(1, 0, 2).reshape(SLAB, C)[:NPC]
            curp[perm_slab[c]] = nm
            terms.append(nm)
        curp[N] = 0
        cur_fm.append(terms)
        if _trace[0]:
            a = np.stack(terms).astype(np.float32)
            print(f"[dbg] step {k}: |cur|={np.abs(a).max():.4g} rms={a.std():.4g}")

    # ---------------- phase C ----------------
    nc_c = build_phase_c()
    in_c = []
    for c in range(NCORES):
        zstack = np.empty((2, NW, 128, 4, CW), BF16)
        z0b = np.asarray(res_a[c]["z0bT"])  # [128, 4, 2500], z0 + blin
        zstack[0] = z0b.reshape(128, 4, NW, CW).transpose(2, 0, 1, 3)
        zstack[1] = _fmajor_w(cur_fm[0][c])
        z8stack = np.empty((3, NW, 128, 4, CW), F8)
        for k in range(1, 4):
            z8stack[k - 1] = _fmajor_w(cur_fm[k][c].astype(F8))
        xw = np.ascontiguousarray(
            xT_cores[c].reshape(128, 4, NW, CW).transpose(2, 0, 1, 3))
        in_c.append(dict(zT=zstack, z8T=z8stack, thT=theta_out[c], xT=xw,
                         idn=ident, id8=ident8,
                         cb2=np.full((128, 1), HALFPI, np.float32),
                         zb2=np.zeros((128, 1), np.float32)))
    res_c = _run(nc_c, in_c, "C")
    ssum = np.zeros(C, np.float64)
    ssq = np.zeros(C, np.float64)
    for c in range(NCORES):
        st = np.asarray(res_c[c]["stats"]).astype(np.float64)  # [128, 8, NW]
        for q in range(4):
            ssum[q * 128:(q + 1) * 128] += st[:, 2 * q, :].sum(axis=1)
            ssq[q * 128:(q + 1) * 128] += st[:, 2 * q + 1, :].sum(axis=1)
    mean = ssum / N
    var = ssq / N - mean ** 2
    if _trace[0]:
        print(f"[dbg] mean range [{mean.min():.4g},{mean.max():.4g}] "
              f"var range [{var.min():.4g},{var.max():.4g}]")
    scale = (gap.astype(np.float64) / np.sqrt(var + EPS)).astype(np.float32)
    shift = (bep.astype(np.float64) - mean * scale).astype(np.float32)

    # ---------------- phase D ----------------
    nc_d = build_phase_d()
    scb = np.ascontiguousarray(scale.reshape(4, 128, 1).transpose(1, 0, 2))
    shb = np.ascontiguousarray(shift.reshape(4, 128, 1).transpose(1, 0, 2))
    in_d = [dict(hbnT=np.asarray(res_c[c]["hbnT"]), sc=scb, sh=shb)
            for c in range(NCORES)]
    res_d = _run(nc_d, in_d, "D")

    out = np.empty((N, C), np.float32)
    for c in range(NCORES):
        o = np.asarray(res_d[c]["outT"])  # [NW, 128, 4, CW] bf16
        op = (o.transpose(2, 1, 0, 3).reshape(C, NPC).T.astype(np.float32))
        out[perm_slab[c][:, None], pf[None, :]] = op
    return out
